# revision 6
# baseline (speedup 1.0000x reference)
"""NeighborSample Trainium2 kernel — pure-DMA, 3-ring streaming (v5).

Input  x:   (8, 64, 64, 192) f32
Output:     (8*64*64, 5, 5, 192) f32 — out[b*4096 + h*64 + w, i, j, c] =
            x[b, h+i-2, w+j-2, c] (zero-padded).

Trace analysis of v1 (two rings, 3840 B descriptors) and v3 (one ring,
38400 B descriptors from materialized patches) showed:
  - a single SDMA engine moves a 3840 B descriptor in ~149 ns (25.8 GB/s)
    when its descriptor feed is deep, because the engine pipelines the SBUF
    read of descriptor n+1 with the HBM write of descriptor n;
  - one large descriptor serializes its own read/write -> only ~14.5 GB/s;
  - v1 averaged 17.5 GB/s/engine because the feed had gaps (2 rings, uneven
    14/15-row segments with <16-engine fan-out, serial zero-row stores).

v5 therefore keeps the v1 dataflow (sliding-window source APs, 3840 B
descriptors, one descriptor per (h, w, i)) and fixes the feed:
  - host-pads x to xp [68, 68, 192]: buf partitions hold padded rows, so
    every store DMA covers a full uniform [32 h x 32 w x 960] block with
    16-engine fan-out; no memsets, no zero-row stores, no segment tails.
  - bufL/bufR [68 partitions x 6912]: padded rows, left half cols [0, 36),
    right half cols [32, 68). Loaded by 4 DMAs (27648 B descriptors).
  - 20 store DMAs (2 halves x 5 i x 2 h-segments), round-robined over THREE
    descriptor-generation rings: SP HWDGE, ACT HWDGE, Pool SWDGE. Three
    independent streams keep all 16 engines fed continuously.
  - no synchronization at all between stores; each ring waits once for the
    4 loads, then streams.
"""

import sys

for _p in ("/opt/trn_rl_repo",):
    if _p not in sys.path:
        sys.path.insert(0, _p)

import numpy as np

import concourse.bass as bass
import concourse.mybir as mybir
from concourse.bass_utils import run_bass_kernel_spmd

B = 8
H = W = 64
C = 192
K = 5
PAD = 2
HALF = 32                # w positions per half
COLS = 36                # cols per half buffer
ROW = COLS * C           # 6912 elems per buf partition
PROWS = H + 2 * PAD      # 68 padded rows
PXROW = PROWS * C        # 13056 elems per padded-input row
WIN = K * C              # 960 (3840 B descriptor)
OUT_W = K * K * C        # 4800
OUT_H = W * OUT_W        # 307200
HSEG = 32                # h rows per store DMA


def build_nc() -> bass.Bass:
    nc = bass.Bass()
    xp = nc.declare_dram_parameter(
        "xp", [PROWS, PROWS, C], mybir.dt.float32, isOutput=False
    )
    out = nc.declare_dram_parameter(
        "out", [H, W, K, K, C], mybir.dt.float32, isOutput=True
    )

    with (
        nc.Block() as block,
        nc.semaphore("lm0") as lm0,
        nc.semaphore("lm1") as lm1,
        nc.semaphore("lt0") as lt0,
        nc.semaphore("lt1") as lt1,
        nc.semaphore("sS") as sS,
        nc.semaphore("sA") as sA,
        nc.semaphore("sP") as sP,
        nc.sbuf_tensor("bufL", [128, ROW], mybir.dt.float32) as bufL,
        nc.sbuf_tensor("bufR", [128, ROW], mybir.dt.float32) as bufR,
    ):
        bufs = [bufL, bufR]
        load_sems = [lm0, lm1, lt0, lt1]

        # all stores on the one SWDGE ring: exclusive queue ownership keeps
        # every engine on back-to-back descriptors (149 ns fast path)
        jobs = [
            (s, i, g) for g in range(2) for i in range(K) for s in range(2)
        ]

        def emit_loads(eng, which):
            # which: 0 -> left main+tail, 1 -> right main+tail
            s = which
            col0 = s * HALF * C
            eng.dma_start(
                out=bass.AP(bufs[s], 0, [[ROW, 64], [1, ROW]]),
                in_=bass.AP(xp, col0, [[PXROW, 64], [1, ROW]]),
            ).then_inc(load_sems[s], 16)
            eng.dma_start(
                out=bass.AP(bufs[s], 64 * ROW, [[ROW, 4], [1, ROW]]),
                in_=bass.AP(xp, 64 * PXROW + col0, [[PXROW, 4], [1, ROW]]),
            ).then_inc(load_sems[2 + s], 16)

        def emit_stores(eng, my_sem, my_jobs):
            for sem in load_sems:
                eng.wait_ge(sem, 16)
            for s, i, g in my_jobs:
                eng.dma_start(
                    out=bass.AP(
                        out,
                        g * HSEG * OUT_H + s * HALF * OUT_W + i * WIN,
                        [[OUT_H, HSEG], [OUT_W, HALF], [1, WIN]],
                    ),
                    in_=bass.AP(
                        bufs[s],
                        (i + g * HSEG) * ROW,
                        [[ROW, HSEG], [C, HALF], [1, WIN]],
                    ),
                ).then_inc(my_sem, 16)
            eng.wait_ge(my_sem, 16 * len(my_jobs))

        @block.sync
        def _(sync):
            emit_loads(sync, 0)
            sync.wait_ge(sP, 16 * len(jobs))

        @block.scalar
        def _(scalar):
            emit_loads(scalar, 1)
            scalar.wait_ge(sP, 16 * len(jobs))

        @block.gpsimd
        def _(gpsimd):
            emit_stores(gpsimd, sP, jobs)

    return nc


_NC_CACHE = None


def make_in_maps(x):
    return [
        {"xp": np.pad(x[i], ((PAD, PAD), (PAD, PAD), (0, 0)))} for i in range(B)
    ]


def kernel(x) -> np.ndarray:
    global _NC_CACHE
    x = np.asarray(x, dtype=np.float32)
    assert x.shape == (B, H, W, C), x.shape
    if _NC_CACHE is None:
        _NC_CACHE = build_nc()
    in_maps = make_in_maps(x)
    res = run_bass_kernel_spmd(_NC_CACHE, in_maps, list(range(B)))
    outs = [res.results[i]["out"].reshape(H * W, K, K, C) for i in range(B)]
    return np.concatenate(outs, axis=0)
